# revision 1
# baseline (speedup 1.0000x reference)
"""NT-Xent loss kernel for Trainium2 (8 NeuronCores, data-parallel over N).

Inputs: zis, zjs [N=128, B=256, D=128] fp32.
Per sample: reps = concat(zjs[n], zis[n]) -> [512, 128]; cosine similarity
matrix S = normalize(reps) @ normalize(reps).T; loss contribution per row k:
logsumexp_{j!=k}(S[k,j]/T) - S[k,(k+B)%2B]/T, with T=0.5.

Device strategy (per core, 16 samples):
  - two 512KB DMAs per 4-sample quad (zjs on the SP HWDGE ring, zis on the
    ACT ring), all issued upfront; a single dma_start spreads across all 16
    SDMA engines, so few big DMAs beat many small ones
  - row sum-of-squares: one fused mul + reduce per sample (DVE)
  - rsqrt = Exp(-0.5*Ln(ssq)) on ACT, batched over sample groups
    ([1,1,2,4,4,4] -- small first groups shorten pipeline fill); one
    preloaded ACT table set serves both Ln and Exp (no table reloads)
  - normalize+cast to bf16 via tensor_scalar_mul (DVE)
  - transpose chunks on PE (transpose mode, bf16) -> That [D=128, 512 rows]
  - sim chunk m = That_m^T @ That on PE (bf16, fp32 accum, N=512) into its
    own PSUM bank
  - exp(2*sim) per chunk on ACT with accum_out: the fused per-partition
    accumulator IS the row sum -> rs_all[:, 4n+m]; the exp values
    themselves go to a write-only scratch
  - pos terms: elementwise product of transposed halves (DVE) + one-hot
    column-sum matmul on PE accumulating into one PSUM bank
Host: lse = log(rs - e^2) in fp64, final reduction and scaling.
"""

import os
import sys

import numpy as np
import ml_dtypes

if "/opt/trn_rl_repo" not in sys.path:
    sys.path.insert(0, "/opt/trn_rl_repo")

N_CORES = 8
N_FULL, B, D = 128, 256, 128
SPC = N_FULL // N_CORES  # samples per core = 16
TWO_B = 2 * B  # 512
N_CHUNKS = 4  # 512 rows / 128 partitions
TEMP = 0.5
GROUPS = [[0], [1], [2, 3], [4, 5, 6, 7], [8, 9, 10, 11], [12, 13, 14, 15]]

_compiled = None


def _build():
    import concourse.bacc as bacc
    import concourse.tile as tile
    import concourse.mybir as mybir

    f32 = mybir.dt.float32
    bf16 = mybir.dt.bfloat16
    AF = mybir.ActivationFunctionType
    OP = mybir.AluOpType

    loop_n = int(os.environ.get("KLOOP", "1"))

    nc = bacc.Bacc(
        "TRN2",
        target_bir_lowering=False,
        debug=False,
        enable_asserts=False,
        num_devices=N_CORES,
    )

    zjs_d = nc.dram_tensor("zjs", [SPC, B, D], f32, kind="ExternalInput")
    zis_d = nc.dram_tensor("zis", [SPC, B, D], f32, kind="ExternalInput")
    oh_d = nc.dram_tensor("ohstrip", [128, 127], bf16, kind="ExternalInput")
    ident_d = nc.dram_tensor("ident", [128, 128], bf16, kind="ExternalInput")
    rs_d = nc.dram_tensor("rs_out", [SPC * N_CHUNKS, TWO_B], f32, kind="ExternalOutput")
    pos_d = nc.dram_tensor("pos_out", [SPC, B], f32, kind="ExternalOutput")

    with tile.TileContext(nc) as tc:
        # One ACT table set covers both Ln and Exp; preloading it here keeps
        # bacc's table-load pass from ping-ponging between the ln-only and
        # exp-only sets (8 reloads x ~1.3us otherwise).
        from concourse.hw_specs import get_activation_tables

        tabs = list(get_activation_tables(nc.m.arch).keys())
        nc.scalar.add_instruction(
            mybir.InstLoadActFuncSet(
                name=nc.get_next_instruction_name(),
                ins=[],
                outs=[],
                act_func_set_id=tabs.index("natural_log_exp_and_others"),
            )
        )

        with (
            tc.tile_pool(name="raw", bufs=4) as rawp,
            tc.tile_pool(name="scratch", bufs=2) as scrp,
            tc.tile_pool(name="grp", bufs=2) as grpp,
            tc.tile_pool(name="rhat", bufs=3) as rhatp,
            tc.tile_pool(name="that", bufs=2) as thatp,
            tc.tile_pool(name="ework", bufs=2) as ep,
            tc.tile_pool(name="singles", bufs=1) as singles,
            tc.tile_pool(name="psim", bufs=3, space="PSUM") as psim_pool,
            tc.tile_pool(name="pt", bufs=1, space="PSUM") as pt_pool,
            tc.tile_pool(name="prs", bufs=1, space="PSUM") as prs_pool,
        ):
            oh_sb = singles.tile([128, 127], bf16)
            nc.sync.dma_start(out=oh_sb, in_=oh_d.ap())
            ident_sb = singles.tile([128, 128], bf16)
            nc.sync.dma_start(out=ident_sb, in_=ident_d.ap())
            def body():
                rs_ps = prs_pool.tile([128, TWO_B], f32, name="rs_ps")
                raw_tiles = {}
                stat_tiles = {}

                def load_quad(q):
                    """One 512KB DMA per source tensor for samples 4q..4q+3;
                    zjs rides the SP HWDGE ring, zis the ACT ring, so both
                    rings stream in parallel and the ~0.6us fixed cost is
                    amortized over 512KB. Layout [p, src, n, c, d] keeps each
                    source's destination region contiguous."""
                    t = rawp.tile(
                        [128, 2, 4, 2, D], f32, tag="quad", name=f"q_{q}"
                    )
                    for h, (src, eng) in enumerate(((zjs_d, nc.sync), (zis_d, nc.scalar))):
                        eng.dma_start(
                            out=t[:, h, :, :, :],
                            in_=src.ap()[4 * q : 4 * q + 4].rearrange(
                                "n (c p) d -> p n c d", p=128
                            ),
                        )
                    raw_tiles[q] = t

                def chunk_ap(n, c):
                    return raw_tiles[n // 4][:, c // 2, n % 4, c % 2, :]

                def group_ssq_tile(gi):
                    t = grpp.tile(
                        [128, len(GROUPS[gi]) * N_CHUNKS],
                        f32,
                        tag="gssq",
                        name=f"gq_{gi}",
                    )
                    stat_tiles[gi] = t

                def ssq_sample(n, gi, k):
                    """Row sum-of-squares per row chunk -> group tile cols."""
                    sample = raw_tiles[n // 4][:, :, n % 4, :, :]
                    sq_scr = scrp.tile(
                        [128, N_CHUNKS, D], f32, tag="sq", name=f"sq_{n}"
                    )
                    nc.vector.tensor_mul(
                        sq_scr.rearrange("p (h c) d -> p h c d", h=2), sample, sample
                    )
                    nc.vector.tensor_reduce(
                        out=stat_tiles[gi][:, k * N_CHUNKS : (k + 1) * N_CHUNKS],
                        in_=sq_scr,
                        axis=mybir.AxisListType.X,
                        op=OP.add,
                    )

                def rsqrt_group(gi):
                    """scale[:, k*4+c] = ssq^-0.5 for group gi."""
                    grp = GROUPS[gi]
                    ssq_t = stat_tiles.pop(gi)
                    nc.vector.tensor_scalar_max(ssq_t, ssq_t, 1e-16)
                    ln_t = grpp.tile(
                        [128, len(grp) * N_CHUNKS], f32, tag="ln", name=f"ln_{gi}"
                    )
                    nc.scalar.activation(out=ln_t, in_=ssq_t, func=AF.Ln)
                    sc_t = grpp.tile(
                        [128, len(grp) * N_CHUNKS], f32, tag="sc", name=f"sc_{gi}"
                    )
                    nc.scalar.activation(out=sc_t, in_=ln_t, func=AF.Exp, scale=-0.5)
                    return sc_t

                deferred = []  # rs/pos matmuls of the previous sample

                def flush_deferred():
                    while deferred:
                        deferred.pop(0)()

                def main_sample(n, k, sc_t):
                    rhat = rhatp.tile(
                        [128, N_CHUNKS, D], bf16, tag="rhat", name=f"rh_{n}"
                    )
                    for c in range(N_CHUNKS):
                        idx = k * N_CHUNKS + c
                        nc.vector.tensor_scalar_mul(
                            rhat[:, c, :], chunk_ap(n, c), sc_t[:, idx : idx + 1]
                        )

                    tpsum = pt_pool.tile([128, N_CHUNKS, 128], bf16, tag="tps")
                    for c in range(N_CHUNKS):
                        nc.tensor.transpose(
                            out=tpsum[:, c, :], in_=rhat[:, c, :], identity=ident_sb
                        )
                    that = thatp.tile([128, N_CHUNKS * 128], bf16, tag="that")
                    nc.vector.tensor_copy(
                        out=that, in_=tpsum.rearrange("p c d -> p (c d)")
                    )

                    # pos terms: product of transposed halves; the one-hot
                    # column-sum matmul is deferred with the rs matmuls
                    pos_scr = scrp.tile([128, B], bf16, tag="pos", name=f"ps_{n}")
                    nc.vector.tensor_mul(pos_scr, that[:, 0:B], that[:, B : 2 * B])

                    sims = [
                        psim_pool.tile([128, 2 * TWO_B], f32, tag="sim", name=f"sA_{n}"),
                        psim_pool.tile([128, 2 * TWO_B], f32, tag="sim", name=f"sB_{n}"),
                    ]
                    for m in range(N_CHUNKS):
                        nc.tensor.matmul(
                            out=sims[m // 2][:, (m % 2) * TWO_B : (m % 2 + 1) * TWO_B],
                            lhsT=that[:, m * 128 : (m + 1) * 128],
                            rhs=that,
                            start=True,
                            stop=True,
                        )
                    flush_deferred()

                    e_sb = ep.tile([128, N_CHUNKS * TWO_B], bf16, tag="e", name=f"e_{n}")
                    for h in range(2):
                        nc.scalar.activation(
                            out=e_sb[:, h * 2 * TWO_B : (h + 1) * 2 * TWO_B],
                            in_=sims[h],
                            func=AF.Exp,
                            scale=1.0 / TEMP,
                        )

                    def emit_rs_pos():
                        for m in range(N_CHUNKS):
                            idx = N_CHUNKS * n + m
                            nc.tensor.matmul(
                                out=rs_ps[0 : SPC * N_CHUNKS, :],
                                lhsT=oh_sb[:, 63 - idx : 127 - idx],
                                rhs=e_sb[:, m * TWO_B : (m + 1) * TWO_B],
                                start=(idx == 0),
                                stop=(idx == SPC * N_CHUNKS - 1),
                                skip_group_check=True,
                            )
                        nc.tensor.matmul(
                            out=rs_ps[64 : 64 + SPC, 0:B],
                            lhsT=oh_sb[:, 63 - n : 63 - n + SPC],
                            rhs=pos_scr,
                            start=(n == 0),
                            stop=(n == SPC - 1),
                            skip_group_check=True,
                        )

                    deferred.append(emit_rs_pos)

                # prologue: all loads upfront (DMA rings run ahead), group 0 prep
                for q in range(SPC // 4):
                    load_quad(q)
                group_ssq_tile(0)
                for k, n in enumerate(GROUPS[0]):
                    ssq_sample(n, 0, k)
                sc_t = rsqrt_group(0)

                for gi, grp in enumerate(GROUPS):
                    nxt = GROUPS[gi + 1] if gi + 1 < len(GROUPS) else None
                    L = len(grp)
                    # spread next group's ssq over this group's early samples
                    prep_slots = [[] for _ in range(L)]
                    if nxt:
                        group_ssq_tile(gi + 1)
                        for j, nn in enumerate(nxt):
                            prep_slots[j % max(L - 1, 1)].append((nn, j))
                    next_sc = None
                    for k, n in enumerate(grp):
                        for nn, j in prep_slots[k]:
                            ssq_sample(nn, gi + 1, j)
                        if nxt and k == max(L - 2, 0):
                            next_sc = rsqrt_group(gi + 1)
                        main_sample(n, k, sc_t)
                    sc_t = next_sc

                flush_deferred()
                rs_sb = singles.tile([SPC * N_CHUNKS, TWO_B], f32, name="rs_sb")
                nc.vector.tensor_copy(out=rs_sb, in_=rs_ps[0 : SPC * N_CHUNKS, :])
                nc.sync.dma_start(out=rs_d.ap(), in_=rs_sb)
                pos_sb = singles.tile([SPC, B], f32, name="pos_sb")
                nc.vector.tensor_copy(out=pos_sb, in_=rs_ps[64 : 64 + SPC, 0:B])
                nc.sync.dma_start(out=pos_d.ap(), in_=pos_sb)

            if loop_n > 1:
                with tc.For_i(0, loop_n, 1):
                    body()
            else:
                body()

    nc.compile()
    return nc


def _host_constants():
    oh = np.zeros((128, 127), dtype=ml_dtypes.bfloat16)
    oh[:, 63] = 1
    ident = np.eye(128, dtype=ml_dtypes.bfloat16)
    return oh, ident


def kernel(zis, zjs):
    global _compiled
    if _compiled is None:
        _compiled = _build()
    nc = _compiled

    from concourse import bass_utils

    zis = np.ascontiguousarray(np.asarray(zis, dtype=np.float32))
    zjs = np.ascontiguousarray(np.asarray(zjs, dtype=np.float32))
    oh, ident = _host_constants()

    in_maps = []
    for c in range(N_CORES):
        sl = slice(c * SPC, (c + 1) * SPC)
        in_maps.append(
            {
                "zjs": np.ascontiguousarray(zjs[sl]),
                "zis": np.ascontiguousarray(zis[sl]),
                "ohstrip": oh,
                "ident": ident,
            }
        )

    res = bass_utils.run_bass_kernel_spmd(nc, in_maps, core_ids=list(range(N_CORES)))

    total_lse = 0.0
    total_pos = 0.0
    diag = np.exp(np.float64(1.0 / TEMP))
    for r in res.results:
        rs = r["rs_out"].astype(np.float64).reshape(SPC, N_CHUNKS, TWO_B).sum(axis=1)
        total_lse += np.log(rs - diag).sum()
        total_pos += r["pos_out"].astype(np.float64).sum()

    # sum_k pos_k over all 512 rows = 2 * sum_pairs (cos/TEMP) = 4 * sum(pos_out)
    loss = (total_lse - (2.0 / TEMP) * total_pos) / TWO_B
    return np.float32(loss)



# revision 11
# speedup vs baseline: 1.0314x; 1.0314x over previous
"""NT-Xent loss kernel for Trainium2 (8 NeuronCores, data-parallel over N).

Inputs: zis, zjs [N=128, B=256, D=128] fp32.
Per sample: reps = concat(zjs[n], zis[n]) -> [512, 128]; cosine similarity
matrix S = normalize(reps) @ normalize(reps).T; loss contribution per row k:
logsumexp_{j!=k}(S[k,j]/T) - S[k,(k+B)%2B]/T, with T=0.5.

Device strategy (per core, 16 samples):
  - two 512KB DMAs per 4-sample quad (zjs on the SP HWDGE ring, zis on the
    ACT ring), all issued upfront; a single dma_start spreads across all 16
    SDMA engines, so few big DMAs beat many small ones
  - row sum-of-squares: one fused mul + reduce per sample (DVE)
  - rsqrt = Exp(-0.5*Ln(ssq)) on ACT, batched over sample groups
    ([1,1,2,4,4,4] -- small first groups shorten pipeline fill); one
    preloaded ACT table set serves both Ln and Exp (no table reloads)
  - normalize+cast to bf16 via tensor_scalar_mul (DVE)
  - transpose chunks on PE (transpose mode, bf16) -> That [D=128, 512 rows]
  - sim chunk m = That_m^T @ That on PE (bf16, fp32 accum, N=512) into its
    own PSUM bank
  - exp(2*sim) per chunk on ACT with accum_out: the fused per-partition
    accumulator IS the row sum -> rs_all[:, 4n+m]; the exp values
    themselves go to a write-only scratch
  - pos terms: elementwise product of transposed halves (DVE) + one-hot
    column-sum matmul on PE accumulating into one PSUM bank
Host: lse = log(rs - e^2) in fp64, final reduction and scaling.
"""

import os
import sys

import numpy as np
import ml_dtypes

if "/opt/trn_rl_repo" not in sys.path:
    sys.path.insert(0, "/opt/trn_rl_repo")

N_CORES = 8
N_FULL, B, D = 128, 256, 128
SPC = N_FULL // N_CORES  # samples per core = 16
TWO_B = 2 * B  # 512
N_CHUNKS = 4  # 512 rows / 128 partitions
TEMP = 0.5
GROUPS = [[0], [1], [2, 3], [4, 5, 6, 7], [8, 9, 10, 11], [12, 13, 14, 15]]

_compiled = None


def _build():
    import concourse.bacc as bacc
    import concourse.tile as tile
    import concourse.mybir as mybir

    f32 = mybir.dt.float32
    bf16 = mybir.dt.bfloat16
    AF = mybir.ActivationFunctionType
    OP = mybir.AluOpType

    loop_n = int(os.environ.get("KLOOP", "1"))

    nc = bacc.Bacc(
        "TRN2",
        target_bir_lowering=False,
        debug=False,
        enable_asserts=False,
        num_devices=N_CORES,
    )

    zjs_d = nc.dram_tensor("zjs", [SPC, B, D], f32, kind="ExternalInput")
    zis_d = nc.dram_tensor("zis", [SPC, B, D], f32, kind="ExternalInput")
    oh_d = nc.dram_tensor("ohstrip", [128, 127], bf16, kind="ExternalInput")
    ident_d = nc.dram_tensor("ident", [128, 128], bf16, kind="ExternalInput")
    rs_d = nc.dram_tensor("rs_out", [SPC * N_CHUNKS, TWO_B], f32, kind="ExternalOutput")
    pos_d = nc.dram_tensor("pos_out", [SPC, B], f32, kind="ExternalOutput")

    with tile.TileContext(nc) as tc:
        # One ACT table set covers both Ln and Exp; preloading it here keeps
        # bacc's table-load pass from ping-ponging between the ln-only and
        # exp-only sets (8 reloads x ~1.3us otherwise).
        from concourse.hw_specs import get_activation_tables

        tabs = list(get_activation_tables(nc.m.arch).keys())
        nc.scalar.add_instruction(
            mybir.InstLoadActFuncSet(
                name=nc.get_next_instruction_name(),
                ins=[],
                outs=[],
                act_func_set_id=tabs.index("natural_log_exp_and_others"),
            )
        )

        with (
            tc.tile_pool(name="raw", bufs=4) as rawp,
            tc.tile_pool(name="scratch", bufs=2) as scrp,
            tc.tile_pool(name="grp", bufs=2) as grpp,
            tc.tile_pool(name="rhat", bufs=3) as rhatp,
            tc.tile_pool(name="that", bufs=2) as thatp,
            tc.tile_pool(name="ework", bufs=2) as ep,
            tc.tile_pool(name="singles", bufs=1) as singles,
            tc.tile_pool(name="psim", bufs=3, space="PSUM") as psim_pool,
            tc.tile_pool(name="pt", bufs=1, space="PSUM") as pt_pool,
            tc.tile_pool(name="prs", bufs=1, space="PSUM") as prs_pool,
        ):
            oh_sb = singles.tile([128, 127], bf16)
            nc.sync.dma_start(out=oh_sb, in_=oh_d.ap())
            ident_sb = singles.tile([128, 128], bf16)
            nc.sync.dma_start(out=ident_sb, in_=ident_d.ap())
            def body():
                rs_ps = prs_pool.tile([128, TWO_B], f32, name="rs_ps")
                raw_tiles = {}
                stat_tiles = {}

                def load_quad(q):
                    """One 512KB DMA per source tensor for samples 4q..4q+3;
                    zjs rides the SP HWDGE ring, zis the ACT ring, so both
                    rings stream in parallel and the ~0.6us fixed cost is
                    amortized over 512KB. Layout [p, src, n, c, d] keeps each
                    source's destination region contiguous."""
                    t = rawp.tile(
                        [128, 2, 4, 2, D], f32, tag="quad", name=f"q_{q}"
                    )
                    for h, (src, eng) in enumerate(((zjs_d, nc.sync), (zis_d, nc.scalar))):
                        eng.dma_start(
                            out=t[:, h, :, :, :],
                            in_=src.ap()[4 * q : 4 * q + 4].rearrange(
                                "n (c p) d -> p n c d", p=128
                            ),
                        )
                    raw_tiles[q] = t

                def chunk_ap(n, c):
                    return raw_tiles[n // 4][:, c // 2, n % 4, c % 2, :]

                def group_ssq_tile(gi):
                    t = grpp.tile(
                        [128, len(GROUPS[gi]) * N_CHUNKS],
                        f32,
                        tag="gssq",
                        name=f"gq_{gi}",
                    )
                    stat_tiles[gi] = t

                def ssq_sample(n, gi, k):
                    """Row sum-of-squares per row chunk -> group tile cols."""
                    sample = raw_tiles[n // 4][:, :, n % 4, :, :]
                    sq_scr = scrp.tile(
                        [128, N_CHUNKS, D], f32, tag="sq", name=f"sq_{n}"
                    )
                    nc.vector.tensor_mul(
                        sq_scr.rearrange("p (h c) d -> p h c d", h=2), sample, sample
                    )
                    nc.vector.tensor_reduce(
                        out=stat_tiles[gi][:, k * N_CHUNKS : (k + 1) * N_CHUNKS],
                        in_=sq_scr,
                        axis=mybir.AxisListType.X,
                        op=OP.add,
                    )

                def rsqrt_group(gi):
                    """scale[:, k*4+c] = ssq^-0.5 for group gi."""
                    grp = GROUPS[gi]
                    ssq_t = stat_tiles.pop(gi)
                    nc.vector.tensor_scalar_max(ssq_t, ssq_t, 1e-16)
                    ln_t = grpp.tile(
                        [128, len(grp) * N_CHUNKS], f32, tag="ln", name=f"ln_{gi}"
                    )
                    nc.scalar.activation(out=ln_t, in_=ssq_t, func=AF.Ln)
                    sc_t = grpp.tile(
                        [128, len(grp) * N_CHUNKS], f32, tag="sc", name=f"sc_{gi}"
                    )
                    nc.scalar.activation(out=sc_t, in_=ln_t, func=AF.Exp, scale=-0.5)
                    return sc_t

                deferred = []  # rs/pos matmuls of the previous sample

                def flush_deferred():
                    while deferred:
                        deferred.pop(0)()

                def main_sample(n, k, sc_t):
                    rhat = rhatp.tile(
                        [128, N_CHUNKS, D], bf16, tag="rhat", name=f"rh_{n}"
                    )
                    for c in range(N_CHUNKS):
                        idx = k * N_CHUNKS + c
                        nc.vector.tensor_scalar_mul(
                            rhat[:, c, :], chunk_ap(n, c), sc_t[:, idx : idx + 1]
                        )

                    tpsum = pt_pool.tile([128, N_CHUNKS, 128], bf16, tag="tps")
                    for c in range(N_CHUNKS):
                        nc.tensor.transpose(
                            out=tpsum[:, c, :], in_=rhat[:, c, :], identity=ident_sb
                        )
                    that = thatp.tile([128, N_CHUNKS * 128], bf16, tag="that")
                    nc.vector.tensor_copy(
                        out=that, in_=tpsum.rearrange("p c d -> p (c d)")
                    )

                    # pos terms: product of transposed halves; the one-hot
                    # column-sum matmul is deferred with the rs matmuls
                    pos_scr = scrp.tile([128, B], bf16, tag="pos", name=f"ps_{n}")
                    nc.vector.tensor_mul(pos_scr, that[:, 0:B], that[:, B : 2 * B])

                    sims = [
                        psim_pool.tile([128, 2 * TWO_B], f32, tag="sim", name=f"sA_{n}"),
                        psim_pool.tile([128, 2 * TWO_B], f32, tag="sim", name=f"sB_{n}"),
                    ]
                    for m in range(N_CHUNKS):
                        nc.tensor.matmul(
                            out=sims[m // 2][:, (m % 2) * TWO_B : (m % 2 + 1) * TWO_B],
                            lhsT=that[:, m * 128 : (m + 1) * 128],
                            rhs=that,
                            start=True,
                            stop=True,
                        )
                    flush_deferred()

                    e_sb = ep.tile([128, N_CHUNKS * TWO_B], bf16, tag="e", name=f"e_{n}")
                    for h in range(2):
                        nc.scalar.activation(
                            out=e_sb[:, h * 2 * TWO_B : (h + 1) * 2 * TWO_B],
                            in_=sims[h],
                            func=AF.Exp,
                            scale=1.0 / TEMP,
                        )

                    def emit_rs_pos():
                        for m in range(N_CHUNKS):
                            idx = N_CHUNKS * n + m
                            nc.tensor.matmul(
                                out=rs_ps[0 : SPC * N_CHUNKS, :],
                                lhsT=oh_sb[:, 63 - idx : 127 - idx],
                                rhs=e_sb[:, m * TWO_B : (m + 1) * TWO_B],
                                start=(idx == 0),
                                stop=(idx == SPC * N_CHUNKS - 1),
                                skip_group_check=True,
                            )
                        nc.tensor.matmul(
                            out=rs_ps[64 : 64 + SPC, 0:B],
                            lhsT=oh_sb[:, 63 - n : 63 - n + SPC],
                            rhs=pos_scr,
                            start=(n == 0),
                            stop=(n == SPC - 1),
                            skip_group_check=True,
                        )

                    deferred.append(emit_rs_pos)

                # prologue: all loads upfront (DMA rings run ahead), group 0 prep
                for q in range(SPC // 4):
                    load_quad(q)
                group_ssq_tile(0)
                for k, n in enumerate(GROUPS[0]):
                    ssq_sample(n, 0, k)
                sc_t = rsqrt_group(0)

                for gi, grp in enumerate(GROUPS):
                    nxt = GROUPS[gi + 1] if gi + 1 < len(GROUPS) else None
                    L = len(grp)
                    # spread next group's ssq over this group's early samples
                    prep_slots = [[] for _ in range(L)]
                    if nxt:
                        group_ssq_tile(gi + 1)
                        for j, nn in enumerate(nxt):
                            prep_slots[j % max(L - 1, 1)].append((nn, j))
                    next_sc = None
                    for k, n in enumerate(grp):
                        for nn, j in prep_slots[k]:
                            ssq_sample(nn, gi + 1, j)
                        if nxt and k == max(L - 2, 0):
                            next_sc = rsqrt_group(gi + 1)
                        main_sample(n, k, sc_t)
                    sc_t = next_sc

                flush_deferred()
                rs_sb = singles.tile([SPC * N_CHUNKS, TWO_B], f32, name="rs_sb")
                nc.vector.tensor_copy(out=rs_sb, in_=rs_ps[0 : SPC * N_CHUNKS, :])
                nc.sync.dma_start(out=rs_d.ap(), in_=rs_sb)
                pos_sb = singles.tile([SPC, B], f32, name="pos_sb")
                nc.vector.tensor_copy(out=pos_sb, in_=rs_ps[64 : 64 + SPC, 0:B])
                nc.sync.dma_start(out=pos_d.ap(), in_=pos_sb)

            if loop_n > 1:
                with tc.For_i(0, loop_n, 1):
                    body()
            else:
                body()

    nc.compile()
    return nc


def _host_constants():
    oh = np.zeros((128, 127), dtype=ml_dtypes.bfloat16)
    oh[:, 63] = 1
    ident = np.eye(128, dtype=ml_dtypes.bfloat16)
    return oh, ident


def kernel(zis, zjs):
    global _compiled
    if _compiled is None:
        _compiled = _build()
    nc = _compiled

    from concourse import bass_utils

    zis = np.ascontiguousarray(np.asarray(zis, dtype=np.float32))
    zjs = np.ascontiguousarray(np.asarray(zjs, dtype=np.float32))
    oh, ident = _host_constants()

    in_maps = []
    for c in range(N_CORES):
        sl = slice(c * SPC, (c + 1) * SPC)
        in_maps.append(
            {
                "zjs": np.ascontiguousarray(zjs[sl]),
                "zis": np.ascontiguousarray(zis[sl]),
                "ohstrip": oh,
                "ident": ident,
            }
        )

    res = bass_utils.run_bass_kernel_spmd(nc, in_maps, core_ids=list(range(N_CORES)))

    total_lse = 0.0
    total_pos = 0.0
    diag = np.exp(np.float64(1.0 / TEMP))
    for r in res.results:
        rs = r["rs_out"].astype(np.float64).reshape(SPC, N_CHUNKS, TWO_B).sum(axis=1)
        total_lse += np.log(rs - diag).sum()
        total_pos += r["pos_out"].astype(np.float64).sum()

    # sum_k pos_k over all 512 rows = 2 * sum_pairs (cos/TEMP) = 4 * sum(pos_out)
    loss = (total_lse - (2.0 / TEMP) * total_pos) / TWO_B
    return np.float32(loss)

